# revision 1
# baseline (speedup 1.0000x reference)
"""Mixtral sparse MoE block on 8 Trainium2 NeuronCores.

Strategy (per sharding hint): expert parallelism. E=8 experts, 8 cores,
one expert per core. The router (gate matmul + top-2 + softmax) is tiny
(33 MFLOP vs 51.6 GFLOP of expert work) and data-dependent, so it runs
on host as part of the token dispatch: tokens are gathered per expert,
padded to a common length, each core runs its expert's SwiGLU MLP over
its tokens in bf16 (fp32 PSUM accumulation), and the host scatter-adds
the weighted per-expert outputs back (combine).

Device layout: features on partitions, tokens on the free dim.
  up[i,t]   = sum_h W1[h,i] * xT[h,t]   (lhsT = W1 tile, rhs = xT tile)
  gate[i,t] = sum_h W3[h,i] * xT[h,t]
  act[i,t]  = silu(up) * gate           (ACT silu + DVE mul, -> bf16)
  out[h,t]  = sum_i W2[i,h] * act[i,t]
No on-chip transposes needed anywhere.
"""

import numpy as np
import ml_dtypes

import bass_rust
import concourse.bass as bass
import concourse.mybir as mybir
import concourse.tile as tile
from concourse.bass_utils import run_bass_kernel_spmd
from concourse.tile import ScopedClock


def _enforce_single_wait(nc):
    """The walrus in this image rejects >1 sync-wait per instruction
    ("Too many sync wait commands", CoreV3GenImpl setupSyncWait). Hoist
    extra waits onto same-engine nops inserted just before the offender
    — waiting earlier on the same sequencer is always safe."""
    for f in nc.m.functions:
        for bb in f.blocks:
            insts = bb.instructions
            i = 0
            while i < len(insts):
                inst = insts[i]
                si = inst.sync_info
                if si is not None and len(si.on_wait) > 1:
                    waits = list(si.on_wait)
                    if any(w.wait_reg is not None for w in waits):
                        i += 1
                        continue
                    for j, w in enumerate(waits[:-1]):
                        nop = mybir.InstNoOp(
                            name=f"{inst.name}_hw{j}", ins=[], outs=[])
                        nop.engine = inst.engine
                        nop.sync_info = bass_rust.SyncInfo(
                            on_wait=[w], on_update=[])
                        insts.insert(i, nop)
                        i += 1
                    inst.sync_info = bass_rust.SyncInfo(
                        on_wait=[waits[-1]], on_update=list(si.on_update))
                i += 1

P = 128
H = 1024
I = 2048
E = 8
K = 2

BF16 = mybir.dt.bfloat16
F32 = mybir.dt.float32

# Populated by the last kernel() call so a harness can inspect HW timing.
LAST_RESULTS = None

_NC_CACHE = {}


def _t_chunks(t_pad):
    """Split the token free-dim into matmul chunks <= 512 (one PSUM bank)."""
    if t_pad <= 512:
        return [(0, t_pad)]
    half = (t_pad + 1) // 2
    half = (half + 31) // 32 * 32
    return [(0, half), (half, t_pad - half)]


def _build_nc(t_pad):
    """One expert's SwiGLU MLP over t_pad tokens (SPMD program, all cores)."""
    nc = bass.Bass()
    xT = nc.declare_dram_parameter("xT", [H, t_pad], BF16, isOutput=False)
    w1 = nc.declare_dram_parameter("w1", [H, I], BF16, isOutput=False)
    w3 = nc.declare_dram_parameter("w3", [H, I], BF16, isOutput=False)
    w2 = nc.declare_dram_parameter("w2", [I, H], BF16, isOutput=False)
    outT = nc.declare_dram_parameter("outT", [H, t_pad], F32, isOutput=True)

    HK = H // P    # 8 k-tiles over hidden dim
    IT = I // P    # 16 tiles over intermediate dim
    chunks = _t_chunks(t_pad)
    NW_CH = 4      # load W1/W3 in 4 column chunks of 512 so PE starts early
    WCH = I // NW_CH

    with tile.TileContext(nc) as tc:
        with (
            tc.tile_pool(name="x", bufs=1) as xpool,
            tc.tile_pool(name="wu", bufs=1) as wupool,
            tc.tile_pool(name="wg", bufs=1) as wgpool,
            tc.tile_pool(name="wd", bufs=1) as wdpool,
            tc.tile_pool(name="acts", bufs=1) as actpool,
            tc.tile_pool(name="ps", bufs=2, space="PSUM") as pspool,
            tc.tile_pool(name="ev", bufs=3) as evpool,
        ):
            x_sb = []
            for hk in range(HK):
                t = xpool.tile([P, t_pad], BF16, tag=f"x{hk}", name=f"x{hk}")
                nc.sync.dma_start(out=t[:], in_=xT[hk * P:(hk + 1) * P, :])
                x_sb.append(t)

            # Weights for up/gate, loaded in i-column chunks, chunk-major so
            # the first i-tiles' matmuls unblock after ~1/4 of W1+W3 arrives.
            w1_sb = [wupool.tile([P, I], BF16, tag=f"w1_{hk}", name=f"w1_{hk}")
                     for hk in range(HK)]
            w3_sb = [wgpool.tile([P, I], BF16, tag=f"w3_{hk}", name=f"w3_{hk}")
                     for hk in range(HK)]
            for c in range(NW_CH):
                cs = slice(c * WCH, (c + 1) * WCH)
                for hk in range(HK):
                    hs = slice(hk * P, (hk + 1) * P)
                    nc.sync.dma_start(out=w1_sb[hk][:, cs], in_=w1[hs, cs])
                    nc.sync.dma_start(out=w3_sb[hk][:, cs], in_=w3[hs, cs])

            w2_sb = []
            for it in range(IT):
                t = wdpool.tile([P, H], BF16, tag=f"w2_{it}", name=f"w2_{it}")
                nc.sync.dma_start(out=t[:], in_=w2[it * P:(it + 1) * P, :])
                w2_sb.append(t)

            act_sb = [actpool.tile([P, t_pad], BF16, tag=f"a{it}", name=f"a{it}")
                      for it in range(IT)]

            # PE warmup: dummy matmuls on the first x tile while the weight
            # DMAs stream in, so the HAM clock-gate is at 8/8 when the real
            # stream starts (~3.4us of activity needed).
            wn = min(384, t_pad)
            for wi in range(22):
                w_ps = pspool.tile([P, wn], F32, tag="warm", name=f"warm{wi}")
                nc.tensor.matmul(
                    w_ps[:], x_sb[0][:, 0:P], x_sb[0][:, 0:wn],
                    start=True, stop=True)

            # Phase A: up/gate matmuls + fused silu*gate eviction.
            for it in range(IT):
                isl = slice(it * P, (it + 1) * P)
                for (t0, tn) in chunks:
                    tsl = slice(t0, t0 + tn)
                    up_ps = pspool.tile([P, tn], F32, tag="up", name=f"up{it}_{t0}")
                    gt_ps = pspool.tile([P, tn], F32, tag="gt", name=f"gt{it}_{t0}")
                    for hk in range(HK):
                        nc.tensor.matmul(
                            up_ps[:], w1_sb[hk][:, isl], x_sb[hk][:, tsl],
                            start=(hk == 0), stop=(hk == HK - 1))
                    for hk in range(HK):
                        nc.tensor.matmul(
                            gt_ps[:], w3_sb[hk][:, isl], x_sb[hk][:, tsl],
                            start=(hk == 0), stop=(hk == HK - 1))
                    silu_t = evpool.tile([P, tn], F32, tag="silu", name=f"silu{it}_{t0}")
                    nc.scalar.activation(
                        silu_t[:], up_ps[:], mybir.ActivationFunctionType.Silu)
                    nc.vector.tensor_mul(act_sb[it][:, tsl], silu_t[:], gt_ps[:])

            # Phase B: down projection.
            for h in range(HK):
                hsl = slice(h * P, (h + 1) * P)
                for (t0, tn) in chunks:
                    tsl = slice(t0, t0 + tn)
                    o_ps = pspool.tile([P, tn], F32, tag="o", name=f"o{h}_{t0}")
                    for it in range(IT):
                        nc.tensor.matmul(
                            o_ps[:], w2_sb[it][:, hsl], act_sb[it][:, tsl],
                            start=(it == 0), stop=(it == IT - 1))
                    o_sb = evpool.tile([P, tn], F32, tag="osb", bufs=3,
                                       name=f"osb{h}_{t0}")
                    nc.scalar.copy(o_sb[:], o_ps[:])
                    nc.sync.dma_start(out=outT[hsl, tsl], in_=o_sb[:])

    _enforce_single_wait(nc)
    return nc


def kernel(x, Wg, W1, W2, W3, _trace=False):
    global LAST_RESULTS
    xf = np.asarray(x, dtype=np.float32).reshape(-1, H)
    T = xf.shape[0]

    # --- Host router: top-2 + softmax over the selected pair (fp32) ---
    logits = xf @ np.asarray(Wg, dtype=np.float32)           # (T, E)
    top2 = np.argsort(-logits, axis=-1)[:, :K]               # (T, K)
    v = np.take_along_axis(logits, top2, axis=-1)
    m = v.max(axis=-1, keepdims=True)
    p = np.exp(v - m)
    rw = (p / p.sum(axis=-1, keepdims=True)).astype(np.float32)

    # --- Dispatch: gather tokens per expert, pad to common length ---
    idx_e, wt_e = [], []
    for e in range(E):
        rows, slots = np.nonzero(top2 == e)
        idx_e.append(rows)
        wt_e.append(rw[rows, slots])
    cmax = max(len(r) for r in idx_e)
    t_pad = max(64, (cmax + 7) // 8 * 8)

    if t_pad not in _NC_CACHE:
        _NC_CACHE[t_pad] = _build_nc(t_pad)
    nc = _NC_CACHE[t_pad]

    in_maps = []
    for e in range(E):
        xT_e = np.zeros((H, t_pad), dtype=ml_dtypes.bfloat16)
        xT_e[:, :len(idx_e[e])] = xf[idx_e[e]].T.astype(ml_dtypes.bfloat16)
        in_maps.append({
            "xT": xT_e,
            "w1": np.asarray(W1[e], dtype=ml_dtypes.bfloat16),
            "w3": np.asarray(W3[e], dtype=ml_dtypes.bfloat16),
            "w2": np.asarray(W2[e], dtype=ml_dtypes.bfloat16),
        })

    res = run_bass_kernel_spmd(nc, in_maps, list(range(E)), trace=_trace)
    LAST_RESULTS = res

    # --- Combine: weighted scatter-add of per-expert outputs ---
    out = np.zeros((T, H), dtype=np.float32)
    for e in range(E):
        ne = len(idx_e[e])
        Ye = np.asarray(res.results[e]["outT"], dtype=np.float32)[:, :ne].T
        # rows are unique within one expert (top-2 indices are distinct)
        out[idx_e[e]] += Ye * wt_e[e][:, None]
    return out.reshape(np.asarray(x).shape).astype(np.float32)



# revision 2
# speedup vs baseline: 1.0219x; 1.0219x over previous
"""Mixtral sparse MoE block on 8 Trainium2 NeuronCores.

Strategy (per sharding hint): expert parallelism. E=8 experts, 8 cores,
one expert per core. The router (gate matmul + top-2 + softmax) is tiny
(33 MFLOP vs 51.6 GFLOP of expert work) and data-dependent, so it runs
on host as part of the token dispatch: tokens are gathered per expert,
padded to a common length, each core runs its expert's SwiGLU MLP over
its tokens in bf16 (fp32 PSUM accumulation), and the host scatter-adds
the weighted per-expert outputs back (combine).

Device layout: features on partitions, tokens on the free dim.
  up[i,t]   = sum_h W1[h,i] * xT[h,t]   (lhsT = W1 tile, rhs = xT tile)
  gate[i,t] = sum_h W3[h,i] * xT[h,t]
  act[i,t]  = silu(up) * gate           (ACT silu + DVE mul, -> bf16)
  out[h,t]  = sum_i W2[i,h] * act[i,t]

DMA regime: the HWDGE costs ~610ns of serialized issue per dma_start and
~1KB-per-partition-line transfers run at only ~20GB/s per SDMA engine, so
all tensors are pre-packed on host into [128, *] partition-major blocks
and moved with ~20 large dma_starts (4-16KB lines) instead of ~100 small
ones. Weights for up/gate land in i-column chunks (small chunks first) so
the PE can start real matmuls as early as possible; a memset-backed
warmup keeps the PE HAM clock-gate at 8/8 while the first data streams.
"""

import numpy as np
import ml_dtypes

import bass_rust
import concourse.bass as bass
import concourse.mybir as mybir
import concourse.tile as tile
from concourse.bass_utils import run_bass_kernel_spmd


def _enforce_single_wait(nc):
    """The walrus in this image rejects >1 sync-wait per instruction
    ("Too many sync wait commands", CoreV3GenImpl setupSyncWait). Hoist
    extra waits onto same-engine nops inserted just before the offender
    — waiting earlier on the same sequencer is always safe."""
    for f in nc.m.functions:
        for bb in f.blocks:
            insts = bb.instructions
            i = 0
            while i < len(insts):
                inst = insts[i]
                si = inst.sync_info
                if si is not None and len(si.on_wait) > 1:
                    waits = list(si.on_wait)
                    if any(w.wait_reg is not None for w in waits):
                        i += 1
                        continue
                    for j, w in enumerate(waits[:-1]):
                        nop = mybir.InstNoOp(
                            name=f"{inst.name}_hw{j}", ins=[], outs=[])
                        nop.engine = inst.engine
                        nop.sync_info = bass_rust.SyncInfo(
                            on_wait=[w], on_update=[])
                        insts.insert(i, nop)
                        i += 1
                    inst.sync_info = bass_rust.SyncInfo(
                        on_wait=[waits[-1]], on_update=list(si.on_update))
                i += 1

P = 128
H = 1024
I = 2048
E = 8
K = 2
HK = H // P   # 8 k-tiles over hidden dim
IT = I // P   # 16 tiles over intermediate dim

# i-column chunks for the fused W1|W3 block: small first so the first
# up/gate matmuls unblock after ~0.5MB instead of 4MB.
CHUNKS_I = [(0, 128), (128, 128), (256, 256), (512, 512),
            (1024, 512), (1536, 512)]
W13_COLS = 2 * I * HK          # 32768 packed columns per partition
W2_COLS = I * H // P           # 16384
NWARM = 44

BF16 = mybir.dt.bfloat16
F32 = mybir.dt.float32

# Populated by the last kernel() call so a harness can inspect HW timing.
LAST_RESULTS = None

_NC_CACHE = {}


def _w13_off(it, hk, w):
    """Packed column offset of the [P,P] lhsT tile for W1 (w=0) / W3 (w=1),
    i-tile `it`, k-tile `hk`."""
    base = 0
    for (i0, sz) in CHUNKS_I:
        if i0 <= it * P < i0 + sz:
            return base + hk * (2 * sz) + w * sz + (it * P - i0)
        base += 2 * HK * sz
    raise AssertionError


def _t_chunks(t_pad):
    """Split the token free-dim into matmul chunks <= 512 (one PSUM bank)."""
    if t_pad <= 512:
        return [(0, t_pad)]
    half = (t_pad + 1) // 2
    half = (half + 31) // 32 * 32
    return [(0, half), (half, t_pad - half)]


def _build_nc(t_pad):
    """One expert's SwiGLU MLP over t_pad tokens (SPMD program, all cores)."""
    nc = bass.Bass()
    xd = nc.declare_dram_parameter("xd", [P, HK * t_pad], BF16, isOutput=False)
    w13d = nc.declare_dram_parameter("w13d", [P, W13_COLS], BF16, isOutput=False)
    w2d = nc.declare_dram_parameter("w2d", [P, W2_COLS], BF16, isOutput=False)
    outd = nc.declare_dram_parameter("outd", [P, HK * t_pad], BF16, isOutput=True)

    chunks = _t_chunks(t_pad)

    with tile.TileContext(nc) as tc:
        with (
            tc.tile_pool(name="sb", bufs=1) as sb,
            tc.tile_pool(name="ps", bufs=2, space="PSUM") as pspool,
            tc.tile_pool(name="wm", bufs=1, space="PSUM") as wmpool,
            tc.tile_pool(name="ev", bufs=3) as evpool,
        ):
            warm_sb = sb.tile([P, P], BF16, tag="warm", name="warm")
            nc.vector.memset(warm_sb[:], 0.0)

            x_sb = sb.tile([P, HK * t_pad], BF16, tag="x", name="x")
            nc.sync.dma_start(out=x_sb[:], in_=xd[:, :])

            w13_sb = sb.tile([P, W13_COLS], BF16, tag="w13", name="w13")
            base = 0
            for (i0, sz) in CHUNKS_I:
                n = 2 * HK * sz
                nc.sync.dma_start(out=w13_sb[:, base:base + n],
                                  in_=w13d[:, base:base + n])
                base += n

            w2_sb = sb.tile([P, W2_COLS], BF16, tag="w2", name="w2")
            half = W2_COLS // 2
            nc.sync.dma_start(out=w2_sb[:, :half], in_=w2d[:, :half])
            nc.sync.dma_start(out=w2_sb[:, half:], in_=w2d[:, half:])

            act_sb = sb.tile([P, IT * t_pad], BF16, tag="act", name="act")
            o_acc = sb.tile([P, HK * t_pad], BF16, tag="oacc", name="oacc")

            # PE warmup on the memset tile while the data DMAs stream in,
            # so the HAM clock-gate is at 8/8 when the real stream starts.
            for wi in range(NWARM):
                w_ps = wmpool.tile([P, P], F32, tag="warmps", name=f"wps{wi}")
                nc.tensor.matmul(
                    w_ps[:], warm_sb[:], warm_sb[:], start=True, stop=True)

            # Phase A: up/gate matmuls + fused silu*gate eviction.
            for it in range(IT):
                u_offs = [_w13_off(it, hk, 0) for hk in range(HK)]
                g_offs = [_w13_off(it, hk, 1) for hk in range(HK)]
                for (t0, tn) in chunks:
                    up_ps = pspool.tile([P, tn], F32, tag="up", name=f"up{it}_{t0}")
                    gt_ps = pspool.tile([P, tn], F32, tag="gt", name=f"gt{it}_{t0}")
                    for hk in range(HK):
                        nc.tensor.matmul(
                            up_ps[:], w13_sb[:, u_offs[hk]:u_offs[hk] + P],
                            x_sb[:, hk * t_pad + t0:hk * t_pad + t0 + tn],
                            start=(hk == 0), stop=(hk == HK - 1))
                    for hk in range(HK):
                        nc.tensor.matmul(
                            gt_ps[:], w13_sb[:, g_offs[hk]:g_offs[hk] + P],
                            x_sb[:, hk * t_pad + t0:hk * t_pad + t0 + tn],
                            start=(hk == 0), stop=(hk == HK - 1))
                    silu_t = evpool.tile([P, tn], F32, tag="silu",
                                         name=f"silu{it}_{t0}")
                    nc.scalar.activation(
                        silu_t[:], up_ps[:], mybir.ActivationFunctionType.Silu)
                    nc.vector.tensor_mul(
                        act_sb[:, it * t_pad + t0:it * t_pad + t0 + tn],
                        silu_t[:], gt_ps[:])

            # Phase B: down projection, one output stripe per k-tile of H.
            for h in range(HK):
                for (t0, tn) in chunks:
                    o_ps = pspool.tile([P, tn], F32, tag="o", name=f"o{h}_{t0}")
                    for it in range(IT):
                        off = it * H + h * P
                        nc.tensor.matmul(
                            o_ps[:], w2_sb[:, off:off + P],
                            act_sb[:, it * t_pad + t0:it * t_pad + t0 + tn],
                            start=(it == 0), stop=(it == IT - 1))
                    nc.vector.tensor_copy(
                        o_acc[:, h * t_pad + t0:h * t_pad + t0 + tn], o_ps[:])
                nc.sync.dma_start(
                    out=outd[:, h * t_pad:(h + 1) * t_pad],
                    in_=o_acc[:, h * t_pad:(h + 1) * t_pad])

    _enforce_single_wait(nc)
    return nc


def _pack_w13(W1e, W3e):
    w1r = np.asarray(W1e, dtype=ml_dtypes.bfloat16).reshape(HK, P, I)
    w3r = np.asarray(W3e, dtype=ml_dtypes.bfloat16).reshape(HK, P, I)
    blocks = []
    for (i0, sz) in CHUNKS_I:
        b = np.stack([w1r[:, :, i0:i0 + sz], w3r[:, :, i0:i0 + sz]], axis=2)
        blocks.append(b.transpose(1, 0, 2, 3).reshape(P, -1))
    return np.ascontiguousarray(np.concatenate(blocks, axis=1))


def kernel(x, Wg, W1, W2, W3, _trace=False):
    global LAST_RESULTS
    xf = np.asarray(x, dtype=np.float32).reshape(-1, H)
    T = xf.shape[0]

    # --- Host router: top-2 + softmax over the selected pair (fp32) ---
    logits = xf @ np.asarray(Wg, dtype=np.float32)           # (T, E)
    top2 = np.argsort(-logits, axis=-1)[:, :K]               # (T, K)
    v = np.take_along_axis(logits, top2, axis=-1)
    m = v.max(axis=-1, keepdims=True)
    p = np.exp(v - m)
    rw = (p / p.sum(axis=-1, keepdims=True)).astype(np.float32)

    # --- Dispatch: gather tokens per expert, pad to common length ---
    idx_e, wt_e = [], []
    for e in range(E):
        rows, slots = np.nonzero(top2 == e)
        idx_e.append(rows)
        wt_e.append(rw[rows, slots])
    cmax = max(len(r) for r in idx_e)
    t_pad = max(64, (cmax + 7) // 8 * 8)

    if t_pad not in _NC_CACHE:
        _NC_CACHE[t_pad] = _build_nc(t_pad)
    nc = _NC_CACHE[t_pad]

    in_maps = []
    for e in range(E):
        ne = len(idx_e[e])
        xp = np.zeros((P, HK, t_pad), dtype=ml_dtypes.bfloat16)
        xt = xf[idx_e[e]].T.astype(ml_dtypes.bfloat16)       # (H, ne)
        xp[:, :, :ne] = xt.reshape(HK, P, ne).transpose(1, 0, 2)
        w2p = np.asarray(W2[e], dtype=ml_dtypes.bfloat16).reshape(
            IT, P, H).transpose(1, 0, 2).reshape(P, W2_COLS)
        in_maps.append({
            "xd": np.ascontiguousarray(xp.reshape(P, HK * t_pad)),
            "w13d": _pack_w13(W1[e], W3[e]),
            "w2d": np.ascontiguousarray(w2p),
        })

    res = run_bass_kernel_spmd(nc, in_maps, list(range(E)), trace=_trace)
    LAST_RESULTS = res

    # --- Combine: weighted scatter-add of per-expert outputs ---
    out = np.zeros((T, H), dtype=np.float32)
    for e in range(E):
        ne = len(idx_e[e])
        op = np.asarray(res.results[e]["outd"], dtype=np.float32)
        Ye = op.reshape(P, HK, t_pad).transpose(1, 0, 2).reshape(H, t_pad)
        # rows are unique within one expert (top-2 indices are distinct)
        out[idx_e[e]] += Ye[:, :ne].T * wt_e[e][:, None]
    return out.reshape(np.asarray(x).shape).astype(np.float32)


# revision 5
# speedup vs baseline: 1.0768x; 1.0537x over previous
"""Mixtral sparse MoE block on 8 Trainium2 NeuronCores.

Strategy (per sharding hint): expert parallelism. E=8 experts, 8 cores,
one expert per core. The router (gate matmul + top-2 + softmax) is tiny
(33 MFLOP vs 51.6 GFLOP of expert work) and data-dependent, so it runs
on host as part of the token dispatch: tokens are gathered per expert,
padded to a common length, each core runs its expert's SwiGLU MLP over
its tokens in bf16 (fp32 PSUM accumulation), and the host scatter-adds
the weighted per-expert outputs back (combine).

Device layout: features on partitions, tokens on the free dim.
  up[i,t]   = sum_h W1[h,i] * xT[h,t]   (lhsT = W1 tile, rhs = xT tile)
  gate[i,t] = sum_h W3[h,i] * xT[h,t]
  act[i,t]  = silu(up) * gate           (ACT silu + DVE mul, -> bf16)
  out[h,t]  = sum_i W2[i,h] * act[i,t]

DMA regime: the HWDGE costs ~610ns of serialized issue per dma_start and
~1KB-per-partition-line transfers run at only ~20GB/s per SDMA engine, so
all tensors are pre-packed on host into [128, *] partition-major blocks
and moved with ~20 large dma_starts (4-16KB lines) instead of ~100 small
ones. Weights for up/gate land in i-column chunks (small chunks first) so
the PE can start real matmuls as early as possible; a memset-backed
warmup keeps the PE HAM clock-gate at 8/8 while the first data streams.
"""

import numpy as np
import ml_dtypes

import bass_rust
import concourse.bass as bass
import concourse.mybir as mybir
import concourse.tile as tile
from concourse.bass_utils import run_bass_kernel_spmd


def _enforce_single_wait(nc):
    """The walrus in this image rejects >1 sync-wait per instruction
    ("Too many sync wait commands", CoreV3GenImpl setupSyncWait). Hoist
    extra waits onto same-engine nops inserted just before the offender
    — waiting earlier on the same sequencer is always safe."""
    for f in nc.m.functions:
        for bb in f.blocks:
            insts = bb.instructions
            i = 0
            while i < len(insts):
                inst = insts[i]
                si = inst.sync_info
                if si is not None and len(si.on_wait) > 1:
                    waits = list(si.on_wait)
                    if any(w.wait_reg is not None for w in waits):
                        i += 1
                        continue
                    for j, w in enumerate(waits[:-1]):
                        nop = mybir.InstNoOp(
                            name=f"{inst.name}_hw{j}", ins=[], outs=[])
                        nop.engine = inst.engine
                        nop.sync_info = bass_rust.SyncInfo(
                            on_wait=[w], on_update=[])
                        insts.insert(i, nop)
                        i += 1
                    inst.sync_info = bass_rust.SyncInfo(
                        on_wait=[waits[-1]], on_update=list(si.on_update))
                i += 1

P = 128
H = 1024
I = 2048
E = 8
K = 2
HK = H // P   # 8 k-tiles over hidden dim
IT = I // P   # 16 tiles over intermediate dim

# i-column chunks for the fused W1|W3 block: small first so the first
# up/gate matmuls unblock after ~0.5MB instead of 4MB.
CHUNKS_I = [(0, 128), (128, 128), (256, 256), (512, 512),
            (1024, 512), (1536, 512)]
W13_COLS = 2 * I * HK          # 32768 packed columns per partition
W2_COLS = I * H // P           # 16384
NWARM = 36

BF16 = mybir.dt.bfloat16
F32 = mybir.dt.float32

# Populated by the last kernel() call so a harness can inspect HW timing.
LAST_RESULTS = None

_NC_CACHE = {}


def _w13_off(it, hk, w):
    """Packed column offset of the [P,P] lhsT tile for W1 (w=0) / W3 (w=1),
    i-tile `it`, k-tile `hk`."""
    base = 0
    for (i0, sz) in CHUNKS_I:
        if i0 <= it * P < i0 + sz:
            return base + hk * (2 * sz) + w * sz + (it * P - i0)
        base += 2 * HK * sz
    raise AssertionError


def _t_chunks(t_pad):
    """Split the token free-dim into matmul chunks <= 512 (one PSUM bank)."""
    if t_pad <= 512:
        return [(0, t_pad)]
    half = (t_pad + 1) // 2
    half = (half + 31) // 32 * 32
    return [(0, half), (half, t_pad - half)]


def _build_nc(t_pad):
    """One expert's SwiGLU MLP over t_pad tokens (SPMD program, all cores)."""
    nc = bass.Bass()
    xd = nc.declare_dram_parameter("xd", [P, HK * t_pad], BF16, isOutput=False)
    w13d = nc.declare_dram_parameter("w13d", [P, W13_COLS], BF16, isOutput=False)
    w2d = nc.declare_dram_parameter("w2d", [P, W2_COLS], BF16, isOutput=False)
    wrmd = nc.declare_dram_parameter("wrmd", [P, P], BF16, isOutput=False)
    outd = nc.declare_dram_parameter("outd", [P, HK * t_pad], BF16, isOutput=True)

    chunks = _t_chunks(t_pad)
    # x is packed token-chunk-major: [chunk][hk][tn]
    xbase = [0]
    for (t0, tn) in chunks:
        xbase.append(xbase[-1] + HK * tn)

    def x_ap(x_sb, hk, ci, tn):
        b = xbase[ci] + hk * tn
        return x_sb[:, b:b + tn]

    with tile.TileContext(nc) as tc:
        with (
            tc.tile_pool(name="sb", bufs=1) as sb,
            tc.tile_pool(name="ps", bufs=2, space="PSUM") as pspool,
            tc.tile_pool(name="wm", bufs=2, space="PSUM") as wmpool,
            tc.tile_pool(name="ev", bufs=3) as evpool,
        ):
            warm_sb = sb.tile([P, P], BF16, tag="warm", name="warm")
            nc.sync.dma_start(out=warm_sb[:], in_=wrmd[:, :])

            # First matmul group needs x chunk 0 + w13 chunk 0 (~1.1 MB);
            # everything else streams behind it, w2 last (Phase B only).
            x_sb = sb.tile([P, HK * t_pad], BF16, tag="x", name="x")
            w13_sb = sb.tile([P, W13_COLS], BF16, tag="w13", name="w13")
            w13_lim = []
            base = 0
            for ci, (i0, sz) in enumerate(CHUNKS_I):
                n = 2 * HK * sz
                if ci == 0:
                    nc.sync.dma_start(out=x_sb[:, xbase[0]:xbase[1]],
                                      in_=xd[:, xbase[0]:xbase[1]])
                    nc.sync.dma_start(out=w13_sb[:, base:base + n],
                                      in_=w13d[:, base:base + n])
                    if len(chunks) > 1:
                        nc.sync.dma_start(out=x_sb[:, xbase[1]:xbase[2]],
                                          in_=xd[:, xbase[1]:xbase[2]])
                else:
                    nc.sync.dma_start(out=w13_sb[:, base:base + n],
                                      in_=w13d[:, base:base + n])
                base += n
                w13_lim.append(base)

            w2_sb = sb.tile([P, W2_COLS], BF16, tag="w2", name="w2")
            half = W2_COLS // 2
            nc.sync.dma_start(out=w2_sb[:, :half], in_=w2d[:, :half])
            nc.sync.dma_start(out=w2_sb[:, half:], in_=w2d[:, half:])

            act_sb = sb.tile([P, IT * t_pad], BF16, tag="act", name="act")
            o_acc = sb.tile([P, HK * t_pad], BF16, tag="oacc", name="oacc")

            # PE warmup on the random tile while the data DMAs stream in, so
            # the HAM clock-gate is at 8/8 when the real stream starts (all-
            # zero matmuls don't register as PE activity; random data does).
            for wi in range(NWARM):
                w_ps = wmpool.tile([P, P], F32, tag="warmps", name=f"wps{wi}")
                nc.tensor.matmul(
                    w_ps[:], warm_sb[:], warm_sb[:], start=True, stop=True)

            # Phase A: up/gate matmuls + fused silu*gate eviction.
            for it in range(IT):
                u_offs = [_w13_off(it, hk, 0) for hk in range(HK)]
                g_offs = [_w13_off(it, hk, 1) for hk in range(HK)]
                for ci, (t0, tn) in enumerate(chunks):
                    up_ps = pspool.tile([P, tn], F32, tag="up", name=f"up{it}_{t0}")
                    gt_ps = pspool.tile([P, tn], F32, tag="gt", name=f"gt{it}_{t0}")
                    for hk in range(HK):
                        nc.tensor.matmul(
                            up_ps[:], w13_sb[:, u_offs[hk]:u_offs[hk] + P],
                            x_ap(x_sb, hk, ci, tn),
                            start=(hk == 0), stop=(hk == HK - 1))
                    for hk in range(HK):
                        nc.tensor.matmul(
                            gt_ps[:], w13_sb[:, g_offs[hk]:g_offs[hk] + P],
                            x_ap(x_sb, hk, ci, tn),
                            start=(hk == 0), stop=(hk == HK - 1))
                    silu_t = evpool.tile([P, tn], F32, tag="silu",
                                         name=f"silu{it}_{t0}")
                    nc.scalar.activation(
                        silu_t[:], up_ps[:], mybir.ActivationFunctionType.Silu)
                    nc.vector.tensor_mul(
                        act_sb[:, it * t_pad + t0:it * t_pad + t0 + tn],
                        silu_t[:], gt_ps[:])

            # Phase B: down projection; per-(h,chunk) output DMA on the
            # scalar HWDGE ring (idle in Phase B) to keep the tail short.
            for h in range(HK):
                for (t0, tn) in chunks:
                    o_ps = pspool.tile([P, tn], F32, tag="o", name=f"o{h}_{t0}")
                    for it in range(IT):
                        off = it * H + h * P
                        nc.tensor.matmul(
                            o_ps[:], w2_sb[:, off:off + P],
                            act_sb[:, it * t_pad + t0:it * t_pad + t0 + tn],
                            start=(it == 0), stop=(it == IT - 1))
                    nc.vector.tensor_copy(
                        o_acc[:, h * t_pad + t0:h * t_pad + t0 + tn], o_ps[:])
                    nc.scalar.dma_start(
                        out=outd[:, h * t_pad + t0:h * t_pad + t0 + tn],
                        in_=o_acc[:, h * t_pad + t0:h * t_pad + t0 + tn])

    _enforce_single_wait(nc)
    return nc


def _pack_w13(W1e, W3e):
    w1r = np.asarray(W1e, dtype=ml_dtypes.bfloat16).reshape(HK, P, I)
    w3r = np.asarray(W3e, dtype=ml_dtypes.bfloat16).reshape(HK, P, I)
    blocks = []
    for (i0, sz) in CHUNKS_I:
        b = np.stack([w1r[:, :, i0:i0 + sz], w3r[:, :, i0:i0 + sz]], axis=2)
        blocks.append(b.transpose(1, 0, 2, 3).reshape(P, -1))
    return np.ascontiguousarray(np.concatenate(blocks, axis=1))


def kernel(x, Wg, W1, W2, W3, _trace=False):
    global LAST_RESULTS
    xf = np.asarray(x, dtype=np.float32).reshape(-1, H)
    T = xf.shape[0]

    # --- Host router: top-2 + softmax over the selected pair (fp32) ---
    logits = xf @ np.asarray(Wg, dtype=np.float32)           # (T, E)
    top2 = np.argsort(-logits, axis=-1)[:, :K]               # (T, K)
    v = np.take_along_axis(logits, top2, axis=-1)
    m = v.max(axis=-1, keepdims=True)
    p = np.exp(v - m)
    rw = (p / p.sum(axis=-1, keepdims=True)).astype(np.float32)

    # --- Dispatch: gather tokens per expert, pad to common length ---
    idx_e, wt_e = [], []
    for e in range(E):
        rows, slots = np.nonzero(top2 == e)
        idx_e.append(rows)
        wt_e.append(rw[rows, slots])
    cmax = max(len(r) for r in idx_e)
    t_pad = max(64, (cmax + 7) // 8 * 8)

    if t_pad not in _NC_CACHE:
        _NC_CACHE[t_pad] = _build_nc(t_pad)
    nc = _NC_CACHE[t_pad]

    chunks = _t_chunks(t_pad)
    rng = np.random.default_rng(0)
    wrm = rng.standard_normal((P, P)).astype(ml_dtypes.bfloat16)
    in_maps = []
    for e in range(E):
        ne = len(idx_e[e])
        xp = np.zeros((P, HK, t_pad), dtype=ml_dtypes.bfloat16)
        xt = xf[idx_e[e]].T.astype(ml_dtypes.bfloat16)       # (H, ne)
        xp[:, :, :ne] = xt.reshape(HK, P, ne).transpose(1, 0, 2)
        # token-chunk-major: [chunk][hk][tn]
        xcm = np.concatenate(
            [xp[:, :, t0:t0 + tn].reshape(P, HK * tn) for (t0, tn) in chunks],
            axis=1)
        w2p = np.asarray(W2[e], dtype=ml_dtypes.bfloat16).reshape(
            IT, P, H).transpose(1, 0, 2).reshape(P, W2_COLS)
        in_maps.append({
            "xd": np.ascontiguousarray(xcm),
            "w13d": _pack_w13(W1[e], W3[e]),
            "w2d": np.ascontiguousarray(w2p),
            "wrmd": wrm,
        })

    res = run_bass_kernel_spmd(nc, in_maps, list(range(E)), trace=_trace)
    LAST_RESULTS = res

    # --- Combine: weighted scatter-add of per-expert outputs ---
    out = np.zeros((T, H), dtype=np.float32)
    for e in range(E):
        ne = len(idx_e[e])
        op = np.asarray(res.results[e]["outd"], dtype=np.float32)
        Ye = op.reshape(P, HK, t_pad).transpose(1, 0, 2).reshape(H, t_pad)
        # rows are unique within one expert (top-2 indices are distinct)
        out[idx_e[e]] += Ye[:, :ne].T * wt_e[e][:, None]
    return out.reshape(np.asarray(x).shape).astype(np.float32)


# revision 12
# speedup vs baseline: 1.0839x; 1.0065x over previous
"""Mixtral sparse MoE block on 8 Trainium2 NeuronCores.

Strategy (per sharding hint): expert parallelism. E=8 experts, 8 cores,
one expert per core. The router (gate matmul + top-2 + softmax) is tiny
(33 MFLOP vs 51.6 GFLOP of expert work) and data-dependent, so it runs
on host as part of the token dispatch: tokens are gathered per expert,
padded to a common length, each core runs its expert's SwiGLU MLP over
its tokens in bf16 (fp32 PSUM accumulation), and the host scatter-adds
the weighted per-expert outputs back (combine).

Device layout: features on partitions, tokens on the free dim.
  up[i,t]   = sum_h W1[h,i] * xT[h,t]   (lhsT = W1 tile, rhs = xT tile)
  gate[i,t] = sum_h W3[h,i] * xT[h,t]
  act[i,t]  = silu(up) * gate           (ACT silu + DVE mul, -> bf16)
  out[h,t]  = sum_i W2[i,h] * act[i,t]

DMA regime: the HWDGE costs ~610ns of serialized issue per dma_start and
~1KB-per-partition-line transfers run at only ~20GB/s per SDMA engine, so
all tensors are pre-packed on host into [128, *] partition-major blocks
and moved with ~20 large dma_starts (4-16KB lines) instead of ~100 small
ones. Weights for up/gate land in i-column chunks (small chunks first) so
the PE can start real matmuls as early as possible; a memset-backed
warmup keeps the PE HAM clock-gate at 8/8 while the first data streams.
"""

import numpy as np
import ml_dtypes

import bass_rust
import concourse.bass as bass
import concourse.mybir as mybir
import concourse.tile as tile
from concourse.bass_utils import run_bass_kernel_spmd


def _enforce_single_wait(nc):
    """The walrus in this image rejects >1 sync-wait per instruction
    ("Too many sync wait commands", CoreV3GenImpl setupSyncWait). Hoist
    extra waits onto same-engine nops inserted just before the offender
    — waiting earlier on the same sequencer is always safe."""
    for f in nc.m.functions:
        for bb in f.blocks:
            insts = bb.instructions
            i = 0
            while i < len(insts):
                inst = insts[i]
                si = inst.sync_info
                if si is not None and len(si.on_wait) > 1:
                    waits = list(si.on_wait)
                    if any(w.wait_reg is not None for w in waits):
                        i += 1
                        continue
                    for j, w in enumerate(waits[:-1]):
                        nop = mybir.InstNoOp(
                            name=f"{inst.name}_hw{j}", ins=[], outs=[])
                        nop.engine = inst.engine
                        nop.sync_info = bass_rust.SyncInfo(
                            on_wait=[w], on_update=[])
                        insts.insert(i, nop)
                        i += 1
                    inst.sync_info = bass_rust.SyncInfo(
                        on_wait=[waits[-1]], on_update=list(si.on_update))
                i += 1

P = 128
H = 1024
I = 2048
E = 8
K = 2
HK = H // P   # 8 k-tiles over hidden dim
IT = I // P   # 16 tiles over intermediate dim

# i-column chunks for the fused W1|W3 block: small first so the first
# up/gate matmuls unblock after ~0.5MB instead of 4MB.
CHUNKS_I = [(0, 128), (128, 128), (256, 256), (512, 512),
            (1024, 512), (1536, 512)]
W13_COLS = 2 * I * HK          # 32768 packed columns per partition
W2_COLS = I * H // P           # 16384
NWARM = 44

BF16 = mybir.dt.bfloat16
F32 = mybir.dt.float32

# Populated by the last kernel() call so a harness can inspect HW timing.
LAST_RESULTS = None

_NC_CACHE = {}


def _w13_off(it, hk, w):
    """Packed column offset of the [P,P] lhsT tile for W1 (w=0) / W3 (w=1),
    i-tile `it`, k-tile `hk`."""
    base = 0
    for (i0, sz) in CHUNKS_I:
        if i0 <= it * P < i0 + sz:
            return base + hk * (2 * sz) + w * sz + (it * P - i0)
        base += 2 * HK * sz
    raise AssertionError


def _t_chunks(t_pad):
    """Split the token free-dim into matmul chunks <= 512 (one PSUM bank)."""
    if t_pad <= 512:
        return [(0, t_pad)]
    half = (t_pad + 1) // 2
    half = (half + 31) // 32 * 32
    return [(0, half), (half, t_pad - half)]


def _build_nc(t_pad):
    """One expert's SwiGLU MLP over t_pad tokens (SPMD program, all cores)."""
    nc = bass.Bass()
    xd = nc.declare_dram_parameter("xd", [P, HK * t_pad], BF16, isOutput=False)
    w13d = nc.declare_dram_parameter("w13d", [P, W13_COLS], BF16, isOutput=False)
    w2d = nc.declare_dram_parameter("w2d", [P, W2_COLS], BF16, isOutput=False)
    outd = nc.declare_dram_parameter("outd", [P, HK * t_pad], BF16, isOutput=True)

    chunks = _t_chunks(t_pad)
    # x is packed token-chunk-major: [chunk][hk][tn]
    xbase = [0]
    for (t0, tn) in chunks:
        xbase.append(xbase[-1] + HK * tn)

    def x_ap(x_sb, hk, ci, tn):
        b = xbase[ci] + hk * tn
        return x_sb[:, b:b + tn]

    with tile.TileContext(nc) as tc:
        with (
            tc.tile_pool(name="sb", bufs=1) as sb,
            tc.tile_pool(name="ps", bufs=2, space="PSUM") as pspool,
            tc.tile_pool(name="wm", bufs=2, space="PSUM") as wmpool,
            tc.tile_pool(name="ev", bufs=3) as evpool,
        ):
            # Warmup operands generated on-chip (no DMA latency): iota data
            # is nonzero/varying so the PE HAM activity monitor counts it.
            # Two distinct tiles — same-tile lhsT+rhs halves the MM rate.
            warm_a = sb.tile([P, P], BF16, tag="warma", name="warma")
            warm_b = sb.tile([P, P], BF16, tag="warmb", name="warmb")
            nc.gpsimd.iota(warm_a[:], pattern=[[1, P]], base=0,
                           channel_multiplier=3,
                           allow_small_or_imprecise_dtypes=True)
            nc.gpsimd.iota(warm_b[:], pattern=[[1, P]], base=7,
                           channel_multiplier=5,
                           allow_small_or_imprecise_dtypes=True)

            # First matmul group needs x chunk 0 + w13 chunk 0 (~1.1 MB);
            # everything else streams behind it, w2 last (Phase B only).
            x_sb = sb.tile([P, HK * t_pad], BF16, tag="x", name="x")
            w13_sb = sb.tile([P, W13_COLS], BF16, tag="w13", name="w13")
            w13_lim = []
            base = 0
            for ci, (i0, sz) in enumerate(CHUNKS_I):
                n = 2 * HK * sz
                if ci == 0:
                    nc.sync.dma_start(out=x_sb[:, xbase[0]:xbase[1]],
                                      in_=xd[:, xbase[0]:xbase[1]])
                    nc.sync.dma_start(out=w13_sb[:, base:base + n],
                                      in_=w13d[:, base:base + n])
                    if len(chunks) > 1:
                        nc.sync.dma_start(out=x_sb[:, xbase[1]:xbase[2]],
                                          in_=xd[:, xbase[1]:xbase[2]])
                else:
                    nc.sync.dma_start(out=w13_sb[:, base:base + n],
                                      in_=w13d[:, base:base + n])
                base += n
                w13_lim.append(base)

            w2_sb = sb.tile([P, W2_COLS], BF16, tag="w2", name="w2")
            half = W2_COLS // 2
            nc.sync.dma_start(out=w2_sb[:, :half], in_=w2d[:, :half])
            nc.sync.dma_start(out=w2_sb[:, half:], in_=w2d[:, half:])

            act_sb = sb.tile([P, IT * t_pad], BF16, tag="act", name="act")
            o_acc = sb.tile([P, HK * t_pad], BF16, tag="oacc", name="oacc")

            # PE warmup on the random tile while the data DMAs stream in, so
            # the HAM clock-gate is at 8/8 when the real stream starts (all-
            # zero matmuls don't register as PE activity; random data does).
            for wi in range(NWARM):
                w_ps = wmpool.tile([P, P], F32, tag="warmps", name=f"wps{wi}")
                nc.tensor.matmul(
                    w_ps[:], warm_a[:], warm_b[:], start=True, stop=True)

            # Phase A: up/gate matmuls + fused silu*gate eviction.
            for it in range(IT):
                u_offs = [_w13_off(it, hk, 0) for hk in range(HK)]
                g_offs = [_w13_off(it, hk, 1) for hk in range(HK)]
                for ci, (t0, tn) in enumerate(chunks):
                    up_ps = pspool.tile([P, tn], F32, tag="up", name=f"up{it}_{t0}")
                    gt_ps = pspool.tile([P, tn], F32, tag="gt", name=f"gt{it}_{t0}")
                    for hk in range(HK):
                        nc.tensor.matmul(
                            up_ps[:], w13_sb[:, u_offs[hk]:u_offs[hk] + P],
                            x_ap(x_sb, hk, ci, tn),
                            start=(hk == 0), stop=(hk == HK - 1))
                    for hk in range(HK):
                        nc.tensor.matmul(
                            gt_ps[:], w13_sb[:, g_offs[hk]:g_offs[hk] + P],
                            x_ap(x_sb, hk, ci, tn),
                            start=(hk == 0), stop=(hk == HK - 1))
                    silu_t = evpool.tile([P, tn], F32, tag="silu",
                                         name=f"silu{it}_{t0}")
                    nc.scalar.activation(
                        silu_t[:], up_ps[:], mybir.ActivationFunctionType.Silu)
                    nc.vector.tensor_mul(
                        act_sb[:, it * t_pad + t0:it * t_pad + t0 + tn],
                        silu_t[:], gt_ps[:])

            # Phase B: down projection; per-(h,chunk) output DMA on the
            # scalar HWDGE ring (idle in Phase B) to keep the tail short.
            # The final unit is split in two so its first half's eviction +
            # DMA (incl. the ~1.5us HBM write receipt) overlaps the last MMs.
            units = [(h, t0, tn) for h in range(HK) for (t0, tn) in chunks]
            lh, lt0, ltn = units.pop()
            ha = (ltn // 2 + 7) // 8 * 8
            units += [(lh, lt0, ha), (lh, lt0 + ha, ltn - ha)]
            for (h, t0, tn) in units:
                o_ps = pspool.tile([P, tn], F32, tag="o", name=f"o{h}_{t0}")
                for it in range(IT):
                    off = it * H + h * P
                    nc.tensor.matmul(
                        o_ps[:], w2_sb[:, off:off + P],
                        act_sb[:, it * t_pad + t0:it * t_pad + t0 + tn],
                        start=(it == 0), stop=(it == IT - 1))
                nc.vector.tensor_copy(
                    o_acc[:, h * t_pad + t0:h * t_pad + t0 + tn], o_ps[:])
                nc.scalar.dma_start(
                    out=outd[:, h * t_pad + t0:h * t_pad + t0 + tn],
                    in_=o_acc[:, h * t_pad + t0:h * t_pad + t0 + tn])

    _enforce_single_wait(nc)
    return nc


def _pack_w13(W1e, W3e):
    w1r = np.asarray(W1e, dtype=ml_dtypes.bfloat16).reshape(HK, P, I)
    w3r = np.asarray(W3e, dtype=ml_dtypes.bfloat16).reshape(HK, P, I)
    blocks = []
    for (i0, sz) in CHUNKS_I:
        b = np.stack([w1r[:, :, i0:i0 + sz], w3r[:, :, i0:i0 + sz]], axis=2)
        blocks.append(b.transpose(1, 0, 2, 3).reshape(P, -1))
    return np.ascontiguousarray(np.concatenate(blocks, axis=1))


def kernel(x, Wg, W1, W2, W3, _trace=False):
    global LAST_RESULTS
    xf = np.asarray(x, dtype=np.float32).reshape(-1, H)
    T = xf.shape[0]

    # --- Host router: top-2 + softmax over the selected pair (fp32) ---
    logits = xf @ np.asarray(Wg, dtype=np.float32)           # (T, E)
    top2 = np.argsort(-logits, axis=-1)[:, :K]               # (T, K)
    v = np.take_along_axis(logits, top2, axis=-1)
    m = v.max(axis=-1, keepdims=True)
    p = np.exp(v - m)
    rw = (p / p.sum(axis=-1, keepdims=True)).astype(np.float32)

    # --- Dispatch: gather tokens per expert, pad to common length ---
    idx_e, wt_e = [], []
    for e in range(E):
        rows, slots = np.nonzero(top2 == e)
        idx_e.append(rows)
        wt_e.append(rw[rows, slots])
    cmax = max(len(r) for r in idx_e)
    t_pad = max(64, (cmax + 7) // 8 * 8)

    if t_pad not in _NC_CACHE:
        _NC_CACHE[t_pad] = _build_nc(t_pad)
    nc = _NC_CACHE[t_pad]

    chunks = _t_chunks(t_pad)
    in_maps = []
    for e in range(E):
        ne = len(idx_e[e])
        xp = np.zeros((P, HK, t_pad), dtype=ml_dtypes.bfloat16)
        xt = xf[idx_e[e]].T.astype(ml_dtypes.bfloat16)       # (H, ne)
        xp[:, :, :ne] = xt.reshape(HK, P, ne).transpose(1, 0, 2)
        # token-chunk-major: [chunk][hk][tn]
        xcm = np.concatenate(
            [xp[:, :, t0:t0 + tn].reshape(P, HK * tn) for (t0, tn) in chunks],
            axis=1)
        w2p = np.asarray(W2[e], dtype=ml_dtypes.bfloat16).reshape(
            IT, P, H).transpose(1, 0, 2).reshape(P, W2_COLS)
        in_maps.append({
            "xd": np.ascontiguousarray(xcm),
            "w13d": _pack_w13(W1[e], W3[e]),
            "w2d": np.ascontiguousarray(w2p),
        })

    res = run_bass_kernel_spmd(nc, in_maps, list(range(E)), trace=_trace)
    LAST_RESULTS = res

    # --- Combine: weighted scatter-add of per-expert outputs ---
    out = np.zeros((T, H), dtype=np.float32)
    for e in range(E):
        ne = len(idx_e[e])
        op = np.asarray(res.results[e]["outd"], dtype=np.float32)
        Ye = op.reshape(P, HK, t_pad).transpose(1, 0, 2).reshape(H, t_pad)
        # rows are unique within one expert (top-2 indices are distinct)
        out[idx_e[e]] += Ye[:, :ne].T * wt_e[e][:, None]
    return out.reshape(np.asarray(x).shape).astype(np.float32)
